# revision 26
# baseline (speedup 1.0000x reference)
"""Trainium2 Bass kernel for nn_AttentionModule (B=32, NC=256, H=W=64, MW=32, E=768).

Data-parallel over batch: 8 NeuronCores x 4 batches each. Per core:
  words_p = W @ words.T + b              (prep, fp32r matmuls; host provides
                                          W.T and words.T pre-transposed)
  scores[m,p] = words_p.T * images       (fp16 matmuls, one [32,512] PSUM tile
                                          per batch; 4 batches packed onto 128
                                          partitions by partition-shifted exp)
  softmax over m:  exp(score/16 + maskbias) -> block-ones matmul replicates the
                   per-batch denominator to all 128 partitions ->
                   reciprocal_approx_fast -> multiply
  weighted[c,p] = words_p @ attn_norm    (fp16 K=32 row-tiled matmuls)
Images stream HBM->SBUF as fp16 (host cast) to halve input bandwidth; outputs
are fp32.  Weighted results stream out per 512-pixel tile; attention maps per
1024-pixel quarter (SWDGE fp16->fp32 cast during DMA).
"""
import numpy as np

import concourse.bacc as bacc
import concourse.mybir as mybir
import concourse.tile as tile
from concourse.bass_utils import run_bass_kernel_spmd

F32 = mybir.dt.float32
F32R = mybir.dt.float32r
F16 = mybir.dt.float16
AF = mybir.ActivationFunctionType

B_CORE = 4        # batches per core
NC_IN = 256       # channels (2 chunks of 128)
P_TOT = 4096      # pixels per image
MW = 32           # words per batch
PT = 512          # pixel tile (one PSUM bank)
NPT = P_TOT // PT
QUAR = 1024       # image-load / attn-out DMA granularity
NEG = -1.0e30

_NC_CACHE = {}
LAST_RESULT = None


def build_nc():
    nc = bacc.Bacc(None, target_bir_lowering=False, debug=False)

    d_img = nc.dram_tensor("images", [B_CORE, NC_IN, P_TOT], F16,
                           kind="ExternalInput")
    # blob_r: wordsT [128,768] | WT [128,1536] | block128 [128,128]
    d_cr = nc.dram_tensor("constr", [128, 2432], F32R, kind="ExternalInput")
    # blob_f: bias2 [128,2] | maskbias [128,1] | ident [128,128]
    d_cf = nc.dram_tensor("constf", [128, 131], F32, kind="ExternalInput")
    d_wgt = nc.dram_tensor("weighted", [B_CORE, NC_IN, P_TOT], F16,
                           kind="ExternalOutput")
    d_att = nc.dram_tensor("attn_out", [B_CORE, MW, P_TOT], F16,
                           kind="ExternalOutput")

    with tile.TileContext(nc) as tc:
        with tc.tile_pool(name="cons", bufs=1) as cons, \
             tc.tile_pool(name="imgs", bufs=16) as imgp, \
             tc.tile_pool(name="expp", bufs=4) as expp, \
             tc.tile_pool(name="recp", bufs=3) as recp, \
             tc.tile_pool(name="anrm", bufs=1) as anrm, \
             tc.tile_pool(name="wstg", bufs=3) as wstg:

            # ---- constant loads (2 DMAs)
            cr = cons.tile([128, 2432], F32R)
            cf = cons.tile([128, 131], F32)
            nc.sync.dma_start(cr[:], d_cr[:])
            nc.sync.dma_start(cf[:], d_cf[:])

            def wordsT(j):
                return cr[:, 128 * j:128 * (j + 1)]

            def WTc(j, k):
                return cr[:, 768 + 256 * j + 128 * k:
                          768 + 256 * j + 128 * (k + 1)]

            blk = cr[:, 2304:2432]
            bias2 = cf[:, 0:2]
            mb = cf[:, 2:3]
            ident = cf[:, 3:131]

            # ---- image loads (SWDGE so issue cost stays off SP/ACT), all
            # issued upfront; [128,2048] fp16 tiles keep 4KB contiguous rows
            img_tiles = {}
            for hh in range(2):
                for b4 in range(B_CORE):
                    for k in range(2):
                        t = imgp.tile([128, 2048], F16, tag="img",
                                      name=f"img{b4}_{k}_{hh}")
                        nc.gpsimd.dma_start(
                            t[:], d_img[b4, 128 * k:128 * (k + 1),
                                        2048 * hh:2048 * (hh + 1)])
                        img_tiles[(b4, k, hh)] = t

            # ---- prep: words_p = W @ words.T + b  (both layouts)
            wp = [cons.tile([128, 128], F32, tag=f"wp{k}", name=f"wp{k}")
                  for k in range(2)]
            wpr = [cons.tile([128, 128], F16, tag=f"wpr{k}", name=f"wpr{k}")
                   for k in range(2)]
            wpT = cons.tile([128, 2, 128], F16)
            with tc.tile_pool(name="pps", bufs=2, space="PSUM") as pps:
                for k in range(2):
                    pwp = pps.tile([128, 128], F32, tag="pwp")
                    for j in range(6):
                        nc.tensor.matmul(
                            pwp[:], WTc(j, k), wordsT(j),
                            start=(j == 0), stop=(j == 5))
                    nc.vector.tensor_scalar_add(wp[k][:], pwp[:],
                                                bias2[:, k:k + 1])
                    ptr = pps.tile([128, 128], F32, tag="ptr")
                    nc.tensor.transpose(ptr[:], wp[k][:], ident)
                    nc.vector.tensor_copy(wpT[:, k, :], ptr[:])
                    nc.vector.tensor_copy(wpr[k][:], wp[k][:])

            blk16 = cons.tile([128, 128], F16)
            nc.vector.tensor_copy(blk16[:], blk.bitcast(F32))
            attn_norm = anrm.tile([128, P_TOT], F16)

            with tc.tile_pool(name="psA", bufs=3, space="PSUM") as psA, \
                 tc.tile_pool(name="psD", bufs=2, space="PSUM") as psD, \
                 tc.tile_pool(name="psW", bufs=3, space="PSUM") as psW:

                wst = {}
                for pt in range(NPT):
                    q, lq = pt // 2, pt % 2

                    # attention scores, one [32,512] psum tile per batch,
                    # packed into 128 partitions by shifted exp writes
                    et = expp.tile([128, PT], F16, tag="et", name=f"et{pt}")
                    for b4 in range(B_CORE):
                        pa = psA.tile([32, PT], F32, tag="pa",
                                      name=f"pa{pt}_{b4}")
                        for k in range(2):
                            nc.tensor.matmul(
                                pa[:],
                                wpr[k][:, 32 * b4:32 * b4 + 32],
                                img_tiles[(b4, k, pt // 4)]
                                [:, PT * (pt % 4):PT * (pt % 4 + 1)],
                                start=(k == 0), stop=(k == 1))
                        nc.scalar.activation(et[32 * b4:32 * b4 + 32, :],
                                             pa[:], AF.Exp,
                                             bias=mb[32 * b4:32 * b4 + 32, :],
                                             scale=0.0625)

                    # per-batch denominator, replicated to all 128 partitions
                    pd = psD.tile([128, PT], F32, tag="pd", name=f"pd{pt}")
                    nc.tensor.matmul(pd[:], blk16[:], et[:],
                                     start=True, stop=True)
                    rt = recp.tile([128, PT], F32, tag="rt", name=f"rt{pt}")
                    nc.vector.reciprocal_approx_fast(rt[:], pd[:])

                    # normalize -> attn_norm slice (fp16)
                    nc.vector.tensor_mul(
                        attn_norm[:, PT * pt:PT * (pt + 1)], et[:], rt[:])

                    # weighted words: K=32 row-tiled fp16 matmuls; batch-inner
                    # order keeps consecutive MMs on different PE row groups so
                    # weight loads overlap in-flight matmuls
                    for b4 in range(B_CORE):
                        for k in range(2):
                            wst[(b4, k)] = wstg.tile(
                                [128, PT], F16, tag=f"wst{b4}_{k}",
                                name=f"wst{b4}_{k}_{pt}")
                    for k in range(2):
                        for b4 in range(B_CORE):
                            pw = psW.tile([128, PT], F32, tag="pw",
                                          name=f"pw{pt}_{b4}_{k}")
                            nc.tensor.matmul(
                                pw[:],
                                wpT[32 * b4:32 * b4 + 32, k, :],
                                attn_norm[32 * b4:32 * b4 + 32,
                                          PT * pt:PT * (pt + 1)],
                                start=True, stop=True,
                                tile_position=(32 * b4, 0))
                            dst = wst[(b4, k)][:]
                            if (pt * 8 + b4 * 2 + k) % 8 < 3:
                                nc.scalar.copy(dst, pw[:])
                            else:
                                nc.vector.tensor_copy(dst, pw[:])

                    # stream outputs per ptile (sync/gpsimd alternate)
                    for b4 in range(B_CORE):
                        for k in range(2):
                            e = nc.sync if k == 0 else nc.gpsimd
                            e.dma_start(
                                d_wgt[b4, 128 * k:128 * (k + 1),
                                      PT * pt:PT * (pt + 1)],
                                wst[(b4, k)][:])
                    if lq == 1:
                        for b4 in range(B_CORE):
                            nc.gpsimd.dma_start(
                                d_att[b4, :, QUAR * q:QUAR * (q + 1)],
                                attn_norm[32 * b4:32 * b4 + 32,
                                          QUAR * q:QUAR * (q + 1)])
    nc.compile()
    return nc


def _host_prep_shared(W, b):
    WT = np.ascontiguousarray(
        W.astype(np.float32).T.reshape(6, 128, 256).transpose(1, 0, 2))
    bias2 = np.ascontiguousarray(b.astype(np.float32).reshape(2, 128).T)
    blk = np.zeros((128, 128), dtype=np.float32)
    for i in range(4):
        blk[32 * i:32 * i + 32, 32 * i:32 * i + 32] = 1.0
    ident = np.eye(128, dtype=np.float32)
    return WT, bias2, blk, ident


def kernel(images, words, mask, W, b):
    global LAST_RESULT
    images = np.asarray(images)
    words = np.asarray(words)
    mask = np.asarray(mask)
    W = np.asarray(W)
    b = np.asarray(b)

    Bf, NCf, Hf, Wf = images.shape
    n_cores = 8
    bpc = Bf // n_cores
    assert bpc == B_CORE

    if "nc" not in _NC_CACHE:
        _NC_CACHE["nc"] = build_nc()
    nc = _NC_CACHE["nc"]

    WT, bias2, blk, ident = _host_prep_shared(W, b)
    imgs = np.ascontiguousarray(
        images.reshape(Bf, NCf, Hf * Wf).astype(np.float16))

    in_maps = []
    for c in range(n_cores):
        sl = slice(bpc * c, bpc * (c + 1))
        wcore = words[sl].astype(np.float32)          # [4, 32, 768]
        wT = np.ascontiguousarray(
            wcore.reshape(128, 768).T.reshape(6, 128, 128).transpose(1, 0, 2))
        mb = np.where(mask[sl].reshape(128), NEG, 0.0).astype(np.float32)
        constr = np.concatenate(
            [wT.reshape(128, 768), WT.reshape(128, 1536), blk], axis=1)
        constf = np.concatenate([bias2, mb[:, None], ident], axis=1)
        in_maps.append(dict(
            images=np.ascontiguousarray(imgs[sl]),
            constr=np.ascontiguousarray(constr),
            constf=np.ascontiguousarray(constf)))

    res = run_bass_kernel_spmd(nc, in_maps, core_ids=list(range(n_cores)))
    LAST_RESULT = res

    weighted = np.concatenate(
        [r["weighted"] for r in res.results], axis=0).astype(np.float32)
    attn = np.concatenate(
        [r["attn_out"] for r in res.results], axis=0).astype(np.float32)
    weighted_words = weighted.reshape(Bf, NCf, Hf, Wf)
    attn_out = attn.reshape(Bf, MW, Hf, Wf)
    return weighted_words, attn_out


# revision 27
# speedup vs baseline: 1.0714x; 1.0714x over previous
"""Trainium2 Bass kernel for nn_AttentionModule (B=32, NC=256, H=W=64, MW=32, E=768).

Data-parallel over batch: 8 NeuronCores x 4 batches each. Per core:
  words_p = W @ words.T + b              (prep, fp32r matmuls; host provides
                                          W.T and words.T pre-transposed)
  scores[m,p] = words_p.T * images       (fp16 matmuls, one [32,512] PSUM tile
                                          per batch; 4 batches packed onto 128
                                          partitions by partition-shifted exp)
  softmax over m:  exp(score/16 + maskbias) -> block-ones matmul replicates the
                   per-batch denominator to all 128 partitions ->
                   reciprocal_approx_fast -> multiply
  weighted[c,p] = words_p @ attn_norm    (fp16 K=32 row-tiled matmuls)
Images stream HBM->SBUF as fp16 (host cast) to halve input bandwidth; outputs
are fp32.  Weighted results stream out per 512-pixel tile; attention maps per
1024-pixel quarter (SWDGE fp16->fp32 cast during DMA).
"""
import numpy as np

import concourse.bacc as bacc
import concourse.mybir as mybir
import concourse.tile as tile
from concourse.bass_utils import run_bass_kernel_spmd

F32 = mybir.dt.float32
F32R = mybir.dt.float32r
F16 = mybir.dt.float16
AF = mybir.ActivationFunctionType

B_CORE = 4        # batches per core
NC_IN = 256       # channels (2 chunks of 128)
P_TOT = 4096      # pixels per image
MW = 32           # words per batch
PT = 512          # pixel tile (one PSUM bank)
NPT = P_TOT // PT
QUAR = 1024       # image-load / attn-out DMA granularity
NEG = -1.0e30

_NC_CACHE = {}
LAST_RESULT = None


def build_nc():
    nc = bacc.Bacc(None, target_bir_lowering=False, debug=False)

    d_img = nc.dram_tensor("images", [B_CORE, NC_IN, P_TOT], F16,
                           kind="ExternalInput")
    # blob_r: wordsT [128,768] | WT [128,1536] | block128 [128,128]
    d_cr = nc.dram_tensor("constr", [128, 2432], F32R, kind="ExternalInput")
    # blob_f: bias2 [128,2] | maskbias [128,1] | ident [128,128]
    d_cf = nc.dram_tensor("constf", [128, 131], F32, kind="ExternalInput")
    d_wgt = nc.dram_tensor("weighted", [B_CORE, NC_IN, P_TOT], F16,
                           kind="ExternalOutput")
    d_att = nc.dram_tensor("attn_out", [B_CORE, MW, P_TOT], F16,
                           kind="ExternalOutput")

    with tile.TileContext(nc) as tc:
        with tc.tile_pool(name="cons", bufs=1) as cons, \
             tc.tile_pool(name="imgs", bufs=32) as imgp, \
             tc.tile_pool(name="expp", bufs=4) as expp, \
             tc.tile_pool(name="recp", bufs=3) as recp, \
             tc.tile_pool(name="anrm", bufs=1) as anrm, \
             tc.tile_pool(name="wstg", bufs=3) as wstg:

            # ---- constant loads (2 DMAs)
            cr = cons.tile([128, 2432], F32R)
            cf = cons.tile([128, 131], F32)
            nc.sync.dma_start(cr[:], d_cr[:])
            nc.sync.dma_start(cf[:], d_cf[:])

            def wordsT(j):
                return cr[:, 128 * j:128 * (j + 1)]

            def WTc(j, k):
                return cr[:, 768 + 256 * j + 128 * k:
                          768 + 256 * j + 128 * (k + 1)]

            blk = cr[:, 2304:2432]
            bias2 = cf[:, 0:2]
            mb = cf[:, 2:3]
            ident = cf[:, 3:131]

            # ---- image loads (SWDGE so issue cost stays off SP/ACT), all
            # issued upfront; the pool admits them as buffers free up
            img_tiles = {}
            for q in range(4):
                for b4 in range(B_CORE):
                    for k in range(2):
                        t = imgp.tile([128, QUAR], F16, tag="img",
                                      name=f"img{b4}_{k}_{q}")
                        nc.gpsimd.dma_start(
                            t[:], d_img[b4, 128 * k:128 * (k + 1),
                                        QUAR * q:QUAR * (q + 1)])
                        img_tiles[(b4, k, q)] = t

            # ---- prep: words_p = W @ words.T + b  (both layouts)
            wp = [cons.tile([128, 128], F32, tag=f"wp{k}", name=f"wp{k}")
                  for k in range(2)]
            wpr = [cons.tile([128, 128], F16, tag=f"wpr{k}", name=f"wpr{k}")
                   for k in range(2)]
            wpT = cons.tile([128, 2, 128], F16)
            with tc.tile_pool(name="pps", bufs=2, space="PSUM") as pps:
                for k in range(2):
                    pwp = pps.tile([128, 128], F32, tag="pwp")
                    for j in range(6):
                        nc.tensor.matmul(
                            pwp[:], WTc(j, k), wordsT(j),
                            start=(j == 0), stop=(j == 5))
                    nc.vector.tensor_scalar_add(wp[k][:], pwp[:],
                                                bias2[:, k:k + 1])
                    ptr = pps.tile([128, 128], F32, tag="ptr")
                    nc.tensor.transpose(ptr[:], wp[k][:], ident)
                    nc.vector.tensor_copy(wpT[:, k, :], ptr[:])
                    nc.vector.tensor_copy(wpr[k][:], wp[k][:])

            blk16 = cons.tile([128, 128], F16)
            nc.vector.tensor_copy(blk16[:], blk.bitcast(F32))
            attn_norm = anrm.tile([128, P_TOT], F16)

            with tc.tile_pool(name="psA", bufs=3, space="PSUM") as psA, \
                 tc.tile_pool(name="psD", bufs=2, space="PSUM") as psD, \
                 tc.tile_pool(name="psW", bufs=3, space="PSUM") as psW:

                wst = {}
                for pt in range(NPT):
                    q, lq = pt // 2, pt % 2

                    # attention scores, one [32,512] psum tile per batch,
                    # packed into 128 partitions by shifted exp writes
                    et = expp.tile([128, PT], F16, tag="et", name=f"et{pt}")
                    for b4 in range(B_CORE):
                        pa = psA.tile([32, PT], F32, tag="pa",
                                      name=f"pa{pt}_{b4}")
                        for k in range(2):
                            nc.tensor.matmul(
                                pa[:],
                                wpr[k][:, 32 * b4:32 * b4 + 32],
                                img_tiles[(b4, k, q)][:, PT * lq:PT * (lq + 1)],
                                start=(k == 0), stop=(k == 1))
                        nc.scalar.activation(et[32 * b4:32 * b4 + 32, :],
                                             pa[:], AF.Exp,
                                             bias=mb[32 * b4:32 * b4 + 32, :],
                                             scale=0.0625)

                    # per-batch denominator, replicated to all 128 partitions
                    pd = psD.tile([128, PT], F32, tag="pd", name=f"pd{pt}")
                    nc.tensor.matmul(pd[:], blk16[:], et[:],
                                     start=True, stop=True)
                    rt = recp.tile([128, PT], F32, tag="rt", name=f"rt{pt}")
                    nc.vector.reciprocal_approx_fast(rt[:], pd[:])

                    # normalize -> attn_norm slice (fp16)
                    nc.vector.tensor_mul(
                        attn_norm[:, PT * pt:PT * (pt + 1)], et[:], rt[:])

                    # weighted words: K=32 row-tiled fp16 matmuls; batch-inner
                    # order keeps consecutive MMs on different PE row groups so
                    # weight loads overlap in-flight matmuls
                    for b4 in range(B_CORE):
                        for k in range(2):
                            wst[(b4, k)] = wstg.tile(
                                [128, PT], F16, tag=f"wst{b4}_{k}",
                                name=f"wst{b4}_{k}_{pt}")
                    for k in range(2):
                        for b4 in range(B_CORE):
                            pw = psW.tile([128, PT], F32, tag="pw",
                                          name=f"pw{pt}_{b4}_{k}")
                            nc.tensor.matmul(
                                pw[:],
                                wpT[32 * b4:32 * b4 + 32, k, :],
                                attn_norm[32 * b4:32 * b4 + 32,
                                          PT * pt:PT * (pt + 1)],
                                start=True, stop=True,
                                tile_position=(32 * b4, 0))
                            dst = wst[(b4, k)][:]
                            if (pt * 8 + b4 * 2 + k) % 8 < 3:
                                nc.scalar.copy(dst, pw[:])
                            else:
                                nc.vector.tensor_copy(dst, pw[:])

                    # stream outputs per ptile (sync/gpsimd alternate)
                    for b4 in range(B_CORE):
                        for k in range(2):
                            e = nc.sync if k == 0 else nc.gpsimd
                            e.dma_start(
                                d_wgt[b4, 128 * k:128 * (k + 1),
                                      PT * pt:PT * (pt + 1)],
                                wst[(b4, k)][:])
                    if lq == 1:
                        for b4 in range(B_CORE):
                            nc.gpsimd.dma_start(
                                d_att[b4, :, QUAR * q:QUAR * (q + 1)],
                                attn_norm[32 * b4:32 * b4 + 32,
                                          QUAR * q:QUAR * (q + 1)])
    nc.compile()
    return nc


def _host_prep_shared(W, b):
    WT = np.ascontiguousarray(
        W.astype(np.float32).T.reshape(6, 128, 256).transpose(1, 0, 2))
    bias2 = np.ascontiguousarray(b.astype(np.float32).reshape(2, 128).T)
    blk = np.zeros((128, 128), dtype=np.float32)
    for i in range(4):
        blk[32 * i:32 * i + 32, 32 * i:32 * i + 32] = 1.0
    ident = np.eye(128, dtype=np.float32)
    return WT, bias2, blk, ident


def kernel(images, words, mask, W, b):
    global LAST_RESULT
    images = np.asarray(images)
    words = np.asarray(words)
    mask = np.asarray(mask)
    W = np.asarray(W)
    b = np.asarray(b)

    Bf, NCf, Hf, Wf = images.shape
    n_cores = 8
    bpc = Bf // n_cores
    assert bpc == B_CORE

    if "nc" not in _NC_CACHE:
        _NC_CACHE["nc"] = build_nc()
    nc = _NC_CACHE["nc"]

    WT, bias2, blk, ident = _host_prep_shared(W, b)
    imgs = np.ascontiguousarray(
        images.reshape(Bf, NCf, Hf * Wf).astype(np.float16))

    in_maps = []
    for c in range(n_cores):
        sl = slice(bpc * c, bpc * (c + 1))
        wcore = words[sl].astype(np.float32)          # [4, 32, 768]
        wT = np.ascontiguousarray(
            wcore.reshape(128, 768).T.reshape(6, 128, 128).transpose(1, 0, 2))
        mb = np.where(mask[sl].reshape(128), NEG, 0.0).astype(np.float32)
        constr = np.concatenate(
            [wT.reshape(128, 768), WT.reshape(128, 1536), blk], axis=1)
        constf = np.concatenate([bias2, mb[:, None], ident], axis=1)
        in_maps.append(dict(
            images=np.ascontiguousarray(imgs[sl]),
            constr=np.ascontiguousarray(constr),
            constf=np.ascontiguousarray(constf)))

    res = run_bass_kernel_spmd(nc, in_maps, core_ids=list(range(n_cores)))
    LAST_RESULT = res

    weighted = np.concatenate(
        [r["weighted"] for r in res.results], axis=0).astype(np.float32)
    attn = np.concatenate(
        [r["attn_out"] for r in res.results], axis=0).astype(np.float32)
    weighted_words = weighted.reshape(Bf, NCf, Hf, Wf)
    attn_out = attn.reshape(Bf, MW, Hf, Wf)
    return weighted_words, attn_out


# revision 28
# speedup vs baseline: 1.1269x; 1.0519x over previous
"""Trainium2 Bass kernel for nn_AttentionModule (B=32, NC=256, H=W=64, MW=32, E=768).

Data-parallel over batch: 8 NeuronCores x 4 batches each. Per core:
  words_p = W @ words.T + b              (prep, fp32r matmuls; host provides
                                          W.T and words.T pre-transposed)
  scores[m,p] = words_p.T * images       (fp16 matmuls, one [32,512] PSUM tile
                                          per batch; 4 batches packed onto 128
                                          partitions by partition-shifted exp)
  softmax over m:  exp(score/16 + maskbias) -> block-ones matmul replicates the
                   per-batch denominator to all 128 partitions ->
                   reciprocal_approx_fast -> multiply
  weighted[c,p] = words_p @ attn_norm    (fp16 K=32 row-tiled matmuls)
Images stream HBM->SBUF as fp16 (host cast) to halve input bandwidth; outputs
are fp32.  Weighted results stream out per 512-pixel tile; attention maps per
1024-pixel quarter (SWDGE fp16->fp32 cast during DMA).
"""
import numpy as np

import concourse.bacc as bacc
import concourse.mybir as mybir
import concourse.tile as tile
from concourse.bass_utils import run_bass_kernel_spmd

F32 = mybir.dt.float32
F32R = mybir.dt.float32r
F16 = mybir.dt.float16
AF = mybir.ActivationFunctionType

B_CORE = 4        # batches per core
NC_IN = 256       # channels (2 chunks of 128)
P_TOT = 4096      # pixels per image
MW = 32           # words per batch
PT = 512          # pixel tile (one PSUM bank)
NPT = P_TOT // PT
QUAR = 1024       # image-load / attn-out DMA granularity
NEG = -1.0e30

_NC_CACHE = {}
LAST_RESULT = None


def build_nc():
    nc = bacc.Bacc(None, target_bir_lowering=False, debug=False)

    d_img = nc.dram_tensor("images", [B_CORE, NC_IN, P_TOT], F16,
                           kind="ExternalInput")
    # blob_r: wordsT [128,768] | WT [128,1536] | block128 [128,128]
    d_cr = nc.dram_tensor("constr", [128, 2432], F32R, kind="ExternalInput")
    # blob_f: bias2 [128,2] | maskbias [128,1] | ident [128,128]
    d_cf = nc.dram_tensor("constf", [128, 131], F32, kind="ExternalInput")
    d_wgt = nc.dram_tensor("weighted", [B_CORE, NC_IN, P_TOT], F16,
                           kind="ExternalOutput")
    d_att = nc.dram_tensor("attn_out", [B_CORE, MW, P_TOT], F16,
                           kind="ExternalOutput")

    with tile.TileContext(nc) as tc:
        with tc.tile_pool(name="cons", bufs=1) as cons, \
             tc.tile_pool(name="imgs", bufs=32) as imgp, \
             tc.tile_pool(name="expp", bufs=4) as expp, \
             tc.tile_pool(name="recp", bufs=3) as recp, \
             tc.tile_pool(name="anrm", bufs=1) as anrm, \
             tc.tile_pool(name="wstg", bufs=3) as wstg:

            # ---- constant loads (2 DMAs)
            cr = cons.tile([128, 2432], F32R)
            cf = cons.tile([128, 131], F32)
            nc.sync.dma_start(cr[:], d_cr[:])
            nc.sync.dma_start(cf[:], d_cf[:])

            def wordsT(j):
                return cr[:, 128 * j:128 * (j + 1)]

            def WTc(j, k):
                return cr[:, 768 + 256 * j + 128 * k:
                          768 + 256 * j + 128 * (k + 1)]

            blk = cr[:, 2304:2432]
            bias2 = cf[:, 0:2]
            mb = cf[:, 2:3]
            ident = cf[:, 3:131]

            # ---- image loads (SWDGE so issue cost stays off SP/ACT), all
            # issued upfront; the pool admits them as buffers free up
            img_tiles = {}
            for q in range(4):
                for b4 in range(B_CORE):
                    for k in range(2):
                        t = imgp.tile([128, QUAR], F16, tag="img",
                                      name=f"img{b4}_{k}_{q}")
                        nc.gpsimd.dma_start(
                            t[:], d_img[b4, 128 * k:128 * (k + 1),
                                        QUAR * q:QUAR * (q + 1)])
                        img_tiles[(b4, k, q)] = t

            # ---- prep: words_p = W @ words.T + b  (both layouts)
            wp = [cons.tile([128, 128], F32, tag=f"wp{k}", name=f"wp{k}")
                  for k in range(2)]
            wpr = [cons.tile([128, 128], F16, tag=f"wpr{k}", name=f"wpr{k}")
                   for k in range(2)]
            wpT = cons.tile([128, 2, 128], F16)
            with tc.tile_pool(name="pps", bufs=2, space="PSUM") as pps:
                for k in range(2):
                    pwp = pps.tile([128, 128], F32, tag="pwp")
                    for j in range(6):
                        nc.tensor.matmul(
                            pwp[:], WTc(j, k), wordsT(j),
                            start=(j == 0), stop=(j == 5))
                    nc.vector.tensor_scalar_add(wp[k][:], pwp[:],
                                                bias2[:, k:k + 1])
                    ptr = pps.tile([128, 128], F32, tag="ptr")
                    nc.tensor.transpose(ptr[:], wp[k][:], ident)
                    nc.vector.tensor_copy(wpT[:, k, :], ptr[:])
                    nc.vector.tensor_copy(wpr[k][:], wp[k][:])

            blk16 = cons.tile([128, 128], F16)
            nc.vector.tensor_copy(blk16[:], blk.bitcast(F32))
            attn_norm = anrm.tile([128, P_TOT], F16)

            with tc.tile_pool(name="psA", bufs=3, space="PSUM") as psA, \
                 tc.tile_pool(name="psD", bufs=2, space="PSUM") as psD, \
                 tc.tile_pool(name="psW", bufs=3, space="PSUM") as psW:

                wst = {}
                for pt in range(NPT):
                    q, lq = pt // 2, pt % 2

                    # attention scores, one [32,512] psum tile per batch,
                    # packed into 128 partitions by shifted exp writes
                    et = expp.tile([128, PT], F16, tag="et", name=f"et{pt}")
                    for b4 in range(B_CORE):
                        pa = psA.tile([32, PT], F32, tag="pa",
                                      name=f"pa{pt}_{b4}")
                        for k in range(2):
                            nc.tensor.matmul(
                                pa[:],
                                wpr[k][:, 32 * b4:32 * b4 + 32],
                                img_tiles[(b4, k, q)][:, PT * lq:PT * (lq + 1)],
                                start=(k == 0), stop=(k == 1))
                        nc.scalar.activation(et[32 * b4:32 * b4 + 32, :],
                                             pa[:], AF.Exp,
                                             bias=mb[32 * b4:32 * b4 + 32, :],
                                             scale=0.0625)

                    # per-batch denominator, replicated to all 128 partitions
                    pd = psD.tile([128, PT], F32, tag="pd", name=f"pd{pt}")
                    nc.tensor.matmul(pd[:], blk16[:], et[:],
                                     start=True, stop=True)
                    rt = recp.tile([128, PT], F32, tag="rt", name=f"rt{pt}")
                    nc.vector.reciprocal_approx_fast(rt[:], pd[:])

                    # normalize -> attn_norm slice (fp16)
                    nc.vector.tensor_mul(
                        attn_norm[:, PT * pt:PT * (pt + 1)], et[:], rt[:])

                    # weighted words: K=32 row-tiled fp16 matmuls; batch-inner
                    # order keeps consecutive MMs on different PE row groups so
                    # weight loads overlap in-flight matmuls
                    for b4 in range(B_CORE):
                        for k in range(2):
                            wst[(b4, k)] = wstg.tile(
                                [128, PT], F16, tag=f"wst{b4}_{k}",
                                name=f"wst{b4}_{k}_{pt}")
                    for k in range(2):
                        for b4 in range(B_CORE):
                            pw = psW.tile([128, PT], F32, tag="pw",
                                          name=f"pw{pt}_{b4}_{k}")
                            nc.tensor.matmul(
                                pw[:],
                                wpT[32 * b4:32 * b4 + 32, k, :],
                                attn_norm[32 * b4:32 * b4 + 32,
                                          PT * pt:PT * (pt + 1)],
                                start=True, stop=True,
                                tile_position=(32 * b4, 0))
                            dst = wst[(b4, k)][:]
                            if (pt * 8 + b4 * 2 + k) % 8 < 3:
                                nc.scalar.copy(dst, pw[:])
                            else:
                                nc.vector.tensor_copy(dst, pw[:])

                    # stream outputs per ptile (sync/gpsimd alternate);
                    # final ptile all on sync so the SWDGE drain ends early
                    for b4 in range(B_CORE):
                        for k in range(2):
                            e = nc.sync if (k == 0 or pt == NPT - 1) \
                                else nc.gpsimd
                            e.dma_start(
                                d_wgt[b4, 128 * k:128 * (k + 1),
                                      PT * pt:PT * (pt + 1)],
                                wst[(b4, k)][:])
                    if lq == 1:
                        for b4 in range(B_CORE):
                            nc.sync.dma_start(
                                d_att[b4, :, QUAR * q:QUAR * (q + 1)],
                                attn_norm[32 * b4:32 * b4 + 32,
                                          QUAR * q:QUAR * (q + 1)])
    nc.compile()
    return nc


def _host_prep_shared(W, b):
    WT = np.ascontiguousarray(
        W.astype(np.float32).T.reshape(6, 128, 256).transpose(1, 0, 2))
    bias2 = np.ascontiguousarray(b.astype(np.float32).reshape(2, 128).T)
    blk = np.zeros((128, 128), dtype=np.float32)
    for i in range(4):
        blk[32 * i:32 * i + 32, 32 * i:32 * i + 32] = 1.0
    ident = np.eye(128, dtype=np.float32)
    return WT, bias2, blk, ident


def kernel(images, words, mask, W, b):
    global LAST_RESULT
    images = np.asarray(images)
    words = np.asarray(words)
    mask = np.asarray(mask)
    W = np.asarray(W)
    b = np.asarray(b)

    Bf, NCf, Hf, Wf = images.shape
    n_cores = 8
    bpc = Bf // n_cores
    assert bpc == B_CORE

    if "nc" not in _NC_CACHE:
        _NC_CACHE["nc"] = build_nc()
    nc = _NC_CACHE["nc"]

    WT, bias2, blk, ident = _host_prep_shared(W, b)
    imgs = np.ascontiguousarray(
        images.reshape(Bf, NCf, Hf * Wf).astype(np.float16))

    in_maps = []
    for c in range(n_cores):
        sl = slice(bpc * c, bpc * (c + 1))
        wcore = words[sl].astype(np.float32)          # [4, 32, 768]
        wT = np.ascontiguousarray(
            wcore.reshape(128, 768).T.reshape(6, 128, 128).transpose(1, 0, 2))
        mb = np.where(mask[sl].reshape(128), NEG, 0.0).astype(np.float32)
        constr = np.concatenate(
            [wT.reshape(128, 768), WT.reshape(128, 1536), blk], axis=1)
        constf = np.concatenate([bias2, mb[:, None], ident], axis=1)
        in_maps.append(dict(
            images=np.ascontiguousarray(imgs[sl]),
            constr=np.ascontiguousarray(constr),
            constf=np.ascontiguousarray(constf)))

    res = run_bass_kernel_spmd(nc, in_maps, core_ids=list(range(n_cores)))
    LAST_RESULT = res

    weighted = np.concatenate(
        [r["weighted"] for r in res.results], axis=0).astype(np.float32)
    attn = np.concatenate(
        [r["attn_out"] for r in res.results], axis=0).astype(np.float32)
    weighted_words = weighted.reshape(Bf, NCf, Hf, Wf)
    attn_out = attn.reshape(Bf, MW, Hf, Wf)
    return weighted_words, attn_out
